# revision 1
# baseline (speedup 1.0000x reference)
"""Multi-head(1) attention kernel for Trainium2, 8 NeuronCores SPMD.

Problem: x[4,4096,1024] @ {Wq,Wk,Wv}[1024,128] -> q,k,v; softmax(q k^T/sqrt(128)) v.

Sharding: core c -> (batch b = c//2, query-half h = c%2).
Each core receives xT = x[b].T (d_model on rows) with the 4096 columns permuted
so that "my" 2048 query rows come first. The core computes kT/v for all 4096
keys (key order is irrelevant under softmax), qT for its first 2048 columns,
and emits outT [128, 2048] = (attention output for its query rows)^T.

On-chip layouts (SBUF is [128 partitions x free]):
  qT, kT : [d_qk=128, seq]   (projection outputs, produced directly by PE)
  v      : [128-row chunk, d_v=128] chunks packed as [128, 4096]
  scoresT chunk: [k-chunk 128, q-block 512] = kT_chunk.T @ qT_block on PE
  U = exp(scoresT * 1/sqrt(dqk)) on ACT (no max subtraction: |scores| <~ 8)
  outT accum in PSUM over 32 k-chunks: out^T += v_chunk.T @ U_chunk
  denominators: DVE accumulates U chunks, PE ones-matmul does partition sum.

All tensors feeding matmuls are float32r (fp32 bits, PE streams at 1 cyc/row
for moving dim >= 256; the BIR verifier requires producers to emit f32r).

SBUF pools stay open for the whole kernel: letting phase-B pools reuse the
xt pool's addresses creates WAR waits against the 8 HWDGE queue semaphores
of the big xt DMAs, overflowing the per-instruction sync-wait limit.
PSUM pools are phase-scoped (only PE writes PSUM -> 1 wait).
"""

import math

import numpy as np

import concourse.bacc as bacc
import concourse.bass as bass
import concourse.mybir as mybir
from concourse.bass import ts
from concourse.masks import make_identity
from concourse.tile import TileContext

P = 128
D_MODEL = 1024
D_QK = 128
B = 4
S_FULL = 4096
N_CORES = 8

F32 = mybir.dt.float32
F32R = mybir.dt.float32r
AF = mybir.ActivationFunctionType

SM_SCALE = 1.0 / math.sqrt(D_QK)

MM_DT = F32R


def _mm(ap):
    return ap


def build_attention(nc: bass.Bass, S: int = S_FULL, SQ: int = S_FULL // 2, repeat: int = 1):
    """Emit the SPMD single-core program. S = #keys, SQ = #queries."""
    assert S % 512 == 0 and SQ % 512 == 0 and D_MODEL % P == 0
    DC = D_MODEL // P  # 8 d_model chunks
    NB = S // 512  # xT column blocks
    QNB = SQ // 512  # of which query blocks
    KC = S // P  # 32 k chunks

    xT = nc.dram_tensor("xT", [D_MODEL, S], MM_DT, kind="ExternalInput").ap()
    wq = nc.dram_tensor("Wq", [D_MODEL, D_QK], MM_DT, kind="ExternalInput").ap()
    bq = nc.dram_tensor("bq", [D_QK], F32, kind="ExternalInput").ap()
    wk = nc.dram_tensor("Wk", [D_MODEL, D_QK], MM_DT, kind="ExternalInput").ap()
    bk = nc.dram_tensor("bk", [D_QK], F32, kind="ExternalInput").ap()
    wv = nc.dram_tensor("Wv", [D_MODEL, D_QK], MM_DT, kind="ExternalInput").ap()
    bv = nc.dram_tensor("bv", [D_QK], F32, kind="ExternalInput").ap()
    outT = nc.dram_tensor("outT", [D_QK, SQ], F32, kind="ExternalOutput").ap()

    with TileContext(nc) as tc:
        lp = nc.allow_low_precision(reason="f32r accumulate of positive exp values")
        lp.__enter__()
        if repeat > 1:
            # benchmarking variant: run the whole kernel `repeat` times on
            # device so wall-clock timing can amortize the dispatch overhead
            loop_cm = tc.For_i(0, repeat, 1)
            loop_cm.__enter__()
        with (
            tc.tile_pool(name="persist", bufs=1) as pp,
            tc.tile_pool(name="xt_pool", bufs=2) as xp,
            tc.tile_pool(name="wka", bufs=3) as wka,
            tc.tile_pool(name="u_pool", bufs=6) as up,
            tc.tile_pool(name="dacc_pool", bufs=2) as dp,
            tc.tile_pool(name="wkb", bufs=3) as wkb,
            # po gets dedicated PSUM banks for the whole kernel: if its banks
            # were reused from phase-A pools, the accumulation-start matmul
            # would carry a bank-WAR wait on top of its RAW wait (2 > limit).
            tc.tile_pool(name="poB", bufs=2, space="PSUM") as poB,
            # one dedicated bank shared (sequentially) by the absorber scratch
            # and the per-q-block dps/bps epilogue tiles, for the same reason
            tc.tile_pool(name="misc", bufs=1, space="PSUM") as mpool,
        ):
            # --- constants ---
            w_sb = {}
            for nm, w in (("q", wq), ("k", wk), ("v", wv)):
                t = pp.tile([P, DC * D_QK], MM_DT, tag=f"w{nm}", name=f"w{nm}_sb")
                nc.sync.dma_start(
                    out=t.rearrange("p (c n) -> p c n", n=D_QK),
                    in_=w.rearrange("(c p) n -> p c n", p=P),
                )
                w_sb[nm] = t
            b_sb = {}
            for nm, b in (("q", bq), ("k", bk), ("v", bv)):
                t = pp.tile([P, 1], F32, tag=f"b{nm}", name=f"b{nm}_sb")
                nc.sync.dma_start(out=t, in_=b.unsqueeze(1))
                b_sb[nm] = t
            ident = pp.tile([P, P], F32, tag="ident")
            make_identity(nc, ident)
            ones_col = pp.tile([P, 1], F32, tag="ones_col")  # lhsT for partition-sum
            nc.gpsimd.memset(ones_col, 1.0)
            ones_col_r = pp.tile([P, 1], MM_DT, tag="ones_col_r")
            nc.vector.tensor_copy(out=ones_col_r, in_=ones_col)
            ones_row = pp.tile([1, P], F32, tag="ones_row")  # lhsT for bcast
            nc.gpsimd.memset(ones_row, 1.0)

            kT = pp.tile([P, S], MM_DT, tag="kT")
            vn = pp.tile([P, S], MM_DT, tag="vn")  # natural-v chunks side by side
            qT = pp.tile([P, SQ], MM_DT, tag="qT")

            # Wait-absorbers: an fp32r matmul lowers to an LDWEIGHTS+MATMUL
            # pair whose LW struct can encode only ONE sync wait. Tile happily
            # attaches 2+ (e.g. weight-DMA lane + xt-DMA lane), which walrus
            # rejects ("Too many sync wait commands"). Tiny PE matmuls reading
            # each DMA'd/POOL-produced tile make the PE observe those
            # semaphores first, so real matmuls need at most one wait.
            babs = wka.tile([P, 1], F32, tag="babs")

            # --- phase A: projections ---
            with (
                tc.tile_pool(name="psA", bufs=3, space="PSUM") as psA,
                tc.tile_pool(name="tpsA", bufs=2, space="PSUM") as tpsA,
            ):
                scr = mpool.tile([1, 1], F32, tag="misc")

                def pe_absorb(ap):
                    a = ap[:, 0:1]
                    if a.dtype != F32:
                        a = a.bitcast(F32)
                    nc.tensor.matmul(scr, a, a, start=True, stop=True)

                for nm in ("q", "k", "v"):
                    pe_absorb(w_sb[nm])  # weight DMA lanes
                    nc.scalar.copy(out=babs, in_=b_sb[nm])  # bias DMA lanes (ACT)
                # POOL-written tiles (each may be the last POOL tick after
                # scheduling, so absorb every one)
                pe_absorb(ident)
                pe_absorb(ones_col)
                pe_absorb(ones_row[0:1, 0:1].broadcast_to([1, 1]))

                for n in range(NB):
                    xt = xp.tile([P, DC * 512], MM_DT, tag="xt")
                    xt3 = xt.rearrange("p (c s) -> p c s", s=512)
                    xT3 = xT[:, ts(n, 512)].rearrange("(c p) s -> p c s", p=P)
                    hc = DC // 2
                    nc.sync.dma_start(out=xt3[:, :hc], in_=xT3[:, :hc])
                    nc.sync.dma_start(out=xt3[:, hc:], in_=xT3[:, hc:])
                    pe_absorb(xt)  # xt DMA lane
                    # kT block
                    kps = psA.tile([P, 512], F32, tag="ps")
                    for c in range(DC):
                        nc.tensor.matmul(
                            kps,
                            _mm(w_sb["k"][:, ts(c, D_QK)]),
                            _mm(xt[:, ts(c, 512)]),
                            start=(c == 0),
                            stop=(c == DC - 1),
                        )
                    nc.vector.tensor_scalar_add(
                        kT[:, ts(n, 512)], kps, b_sb["k"]
                    )
                    # v block: project to vT then PE-transpose to natural chunks
                    vps = psA.tile([P, 512], F32, tag="ps")
                    for c in range(DC):
                        nc.tensor.matmul(
                            vps,
                            _mm(w_sb["v"][:, ts(c, D_QK)]),
                            _mm(xt[:, ts(c, 512)]),
                            start=(c == 0),
                            stop=(c == DC - 1),
                        )
                    vt_tmp = wka.tile([P, 512], MM_DT, tag="vt_tmp")
                    nc.vector.tensor_scalar_add(vt_tmp, vps, b_sb["v"])
                    for j in range(4):
                        tps = tpsA.tile([P, P], F32, tag="tps")
                        nc.tensor.transpose(tps, vt_tmp[:, ts(j, P)].bitcast(F32), ident)
                        nc.scalar.copy(out=vn[:, ts(4 * n + j, P)], in_=tps)
                    # qT block (first SQ columns only)
                    if n < QNB:
                        qps = psA.tile([P, 512], F32, tag="ps")
                        for c in range(DC):
                            nc.tensor.matmul(
                                qps,
                                _mm(w_sb["q"][:, ts(c, D_QK)]),
                                _mm(xt[:, ts(c, 512)]),
                                start=(c == 0),
                                stop=(c == DC - 1),
                            )
                        nc.vector.tensor_scalar_add(
                            qT[:, ts(n, 512)], qps, b_sb["q"]
                        )

            # --- phase B: attention ---
            with tc.tile_pool(name="psB", bufs=2, space="PSUM") as psB:
                KP = KC // 2  # k-chunk pairs; one 1024-wide exp per pair
                for qb in range(QNB):
                    po = poB.tile([P, 512], F32, tag="po")
                    daccs = [
                        dp.tile([P, 512], MM_DT, tag=f"dacc{i}", name=f"dacc{i}_{qb}")
                        for i in range(2)
                    ]
                    dps = mpool.tile([1, 512], F32, tag="misc", name=f"dps_{qb}")
                    us: dict[int, object] = {}
                    # Software-pipelined over k-chunk PAIRS: two scores
                    # matmuls land in the two banks of one [128,1024] PSUM
                    # tile, a single wide exp (ACT fixed cost ~185ns/op is
                    # the phase-B limiter) produces u2, then two PV matmuls.
                    # Denominator: even chunk of each pair accumulates on
                    # DVE, odd chunk rides the PE (ones-matmul into dps).
                    for mp in range(KP + 1):
                        if mp < KP:
                            sps = psB.tile([P, 1024], F32, tag="ps")
                            for h in range(2):
                                nc.tensor.matmul(
                                    sps[:, ts(h, 512)],
                                    _mm(kT[:, ts(2 * mp + h, P)]),
                                    _mm(qT[:, ts(qb, 512)]),
                                    start=True,
                                    stop=True,
                                )
                            u = up.tile([P, 1024], MM_DT, tag="u")
                            nc.scalar.activation(u, sps, AF.Exp, scale=SM_SCALE)
                            us[mp] = u
                            nc.tensor.matmul(
                                dps,
                                _mm(ones_col_r),
                                u[:, ts(1, 512)],
                                start=(mp == 0),
                                stop=False,
                            )
                            if mp < 2:
                                nc.vector.tensor_copy(
                                    out=daccs[mp], in_=u[:, ts(0, 512)]
                                )
                            else:
                                nc.vector.tensor_add(
                                    out=daccs[mp % 2],
                                    in0=daccs[mp % 2],
                                    in1=u[:, ts(0, 512)],
                                )
                        if mp > 0:
                            u_prev = us.pop(mp - 1)
                            for h in range(2):
                                mm = 2 * (mp - 1) + h
                                nc.tensor.matmul(
                                    po,
                                    _mm(vn[:, ts(mm, P)]),
                                    _mm(u_prev[:, ts(h, 512)]),
                                    start=(mm == 0),
                                    stop=(mm == KC - 1),
                                )
                    nc.vector.tensor_add(out=daccs[0], in0=daccs[0], in1=daccs[1])
                    nc.tensor.matmul(
                        dps,
                        _mm(ones_col_r),
                        daccs[0],
                        start=False,
                        stop=True,
                    )
                    rec = wkb.tile([1, 512], F32, tag="rec")
                    nc.vector.reciprocal(out=rec, in_=dps)
                    bps = mpool.tile([P, 512], F32, tag="misc", name=f"bps_{qb}")
                    nc.tensor.matmul(bps, ones_row, rec, start=True, stop=True)
                    bsb = wkb.tile([P, 512], F32, tag="bsb")
                    nc.scalar.copy(out=bsb, in_=bps)
                    # evacuate po on ACT (not DVE): the next po-slot user's WAR
                    # then lands on the ACT sem it already waits on for u.
                    poc = wkb.tile([P, 512], F32, tag="poc")
                    nc.scalar.copy(out=poc, in_=po)
                    fin = wkb.tile([P, 512], F32, tag="fin")
                    nc.vector.tensor_mul(out=fin, in0=poc, in1=bsb)
                    nc.sync.dma_start(out=outT[:, ts(qb, 512)], in_=fin)

        if repeat > 1:
            loop_cm.__exit__(None, None, None)

    return nc


_NC_CACHE: dict = {}


def _get_nc(S: int = S_FULL, SQ: int = S_FULL // 2, repeat: int = 1):
    key = (S, SQ, repeat)
    if key not in _NC_CACHE:
        nc = bacc.Bacc("TRN2", debug=False)
        build_attention(nc, S, SQ, repeat)
        nc.compile()  # splits multi-waits into event semaphores (HW limit)
        _NC_CACHE[key] = nc
    return _NC_CACHE[key]


def make_in_maps(x, Wq, bq, Wk, bk, Wv, bv):
    """Per-core input dicts. Core c = (batch c//2, query-half c%2)."""
    x = np.asarray(x, dtype=np.float32)
    common = {
        "Wq": np.ascontiguousarray(Wq, dtype=np.float32),
        "bq": np.ascontiguousarray(bq, dtype=np.float32),
        "Wk": np.ascontiguousarray(Wk, dtype=np.float32),
        "bk": np.ascontiguousarray(bk, dtype=np.float32),
        "Wv": np.ascontiguousarray(Wv, dtype=np.float32),
        "bv": np.ascontiguousarray(bv, dtype=np.float32),
    }
    in_maps = []
    for c in range(N_CORES):
        b, h = divmod(c, 2)
        xb = x[b]  # [S, D]
        half = S_FULL // 2
        if h == 0:
            perm = xb
        else:
            perm = np.concatenate([xb[half:], xb[:half]], axis=0)
        in_maps.append({"xT": np.ascontiguousarray(perm.T), **common})
    return in_maps


def assemble_output(results):
    """results: list of 8 per-core dicts with 'outT' [128, 2048]."""
    half = S_FULL // 2
    out = np.empty((B, S_FULL, D_QK), dtype=np.float32)
    for c in range(N_CORES):
        b, h = divmod(c, 2)
        out[b, h * half : (h + 1) * half, :] = results[c]["outT"].T
    return out


def kernel(x, Wq, bq, Wk, bk, Wv, bv):
    from concourse.bass_utils import run_bass_kernel_spmd

    nc = _get_nc()
    in_maps = make_in_maps(x, Wq, bq, Wk, bk, Wv, bv)
    res = run_bass_kernel_spmd(nc, in_maps, list(range(N_CORES)))
    return assemble_output(res.results)



# revision 13
# speedup vs baseline: 1.2078x; 1.2078x over previous
"""Multi-head(1) attention kernel for Trainium2, 8 NeuronCores SPMD.

Problem: x[4,4096,1024] @ {Wq,Wk,Wv}[1024,128] -> q,k,v; softmax(q k^T/sqrt(128)) v.

Sharding: core c -> (batch b = c//2, query-half h = c%2).
Each core receives xT = x[b].T (d_model on rows) with the 4096 columns permuted
so that "my" 2048 query rows come first. The core computes kT/v for all 4096
keys (key order is irrelevant under softmax), qT for its first 2048 columns,
and emits outT [128, 2048] = (attention output for its query rows)^T.

v2 (bf16): all matmul operands are bfloat16 (host converts x and W).  This
halves the x HBM traffic (phase A was DMA-bound at f32), keeps PE at
1 cyc/row, and unlocks DVE 2x mode for the softmax-denominator reduction,
which moves entirely off the PE (the f32r version spent ~14.5us of PE time
on ones-matmul partial sums).  PSUM accumulation stays f32.  Expected extra
error from bf16 inputs ~0.5-1% rel, well under the 2e-2 gate.

On-chip layouts (SBUF is [128 partitions x free]):
  qT, kT : [d_qk=128, seq] bf16 (projection outputs, PE -> PSUM -> DVE bias-add)
  v      : [128-row chunk, d_v=128] bf16 chunks packed as [128, 4096]
  scoresT pair: [k-chunk 128, q-block 512] x2 -> one [128,1024] PSUM tile
  U = exp(scoresT * 1/sqrt(dqk)) on ACT, bf16 out (no max subtraction:
      |scores| <~ 8); ACT is the phase-B bottleneck (~1.04us per 1024-wide op)
  outT accum in PSUM over 32 k-chunks: out^T += v_chunk.T @ U_chunk
  denominators: binary tree of bf16 adds over the 16 U pair-tiles on DVE
      (2x mode, ~0.6us/add of [128,1024]), one ones-matmul partition sum.

Phase-B per-q-block engine budget: ACT 16 exps ~16.6us; PE scores+PV 64x512
rows ~14.1us; DVE tree+epilogue ~10us.  Phase A is PE-bound (~36us).

SBUF pools stay open for the whole kernel: letting phase-B pools reuse the
xt pool's addresses creates WAR waits against the 8 HWDGE queue semaphores
of the big xt DMAs, overflowing the per-instruction sync-wait limit.
PSUM pools are phase-scoped (only PE writes PSUM -> 1 wait).
"""

import math

import numpy as np

import concourse.bacc as bacc
import concourse.bass as bass
import concourse.mybir as mybir
from concourse.bass import ts
from concourse.masks import make_identity
from concourse.tile import TileContext

P = 128
D_MODEL = 1024
D_QK = 128
B = 4
S_FULL = 4096
N_CORES = 8

F32 = mybir.dt.float32
F32R = mybir.dt.float32r
BF16 = mybir.dt.bfloat16
AF = mybir.ActivationFunctionType

SM_SCALE = 1.0 / math.sqrt(D_QK)

MM_DT = BF16


def build_attention(nc: bass.Bass, S: int = S_FULL, SQ: int = S_FULL // 2, repeat: int = 1):
    """Emit the SPMD single-core program. S = #keys, SQ = #queries."""
    assert S % 512 == 0 and SQ % 512 == 0 and D_MODEL % P == 0
    DC = D_MODEL // P  # 8 d_model chunks
    NB = S // 512  # xT column blocks
    QNB = SQ // 512  # of which query blocks
    KC = S // P  # 32 k chunks
    KP = KC // 2  # 16 k-chunk pairs

    xT = nc.dram_tensor("xT", [D_MODEL, S], MM_DT, kind="ExternalInput").ap()
    wq = nc.dram_tensor("Wq", [D_MODEL, D_QK], MM_DT, kind="ExternalInput").ap()
    bq = nc.dram_tensor("bq", [D_QK], F32, kind="ExternalInput").ap()
    wk = nc.dram_tensor("Wk", [D_MODEL, D_QK], MM_DT, kind="ExternalInput").ap()
    bk = nc.dram_tensor("bk", [D_QK], F32, kind="ExternalInput").ap()
    wv = nc.dram_tensor("Wv", [D_MODEL, D_QK], MM_DT, kind="ExternalInput").ap()
    bv = nc.dram_tensor("bv", [D_QK], F32, kind="ExternalInput").ap()
    outT = nc.dram_tensor("outT", [D_QK, SQ], F32, kind="ExternalOutput").ap()

    with TileContext(nc) as tc:
        lp = nc.allow_low_precision(reason="bf16 matmuls + bf16 denominator tree")
        lp.__enter__()
        if repeat > 1:
            # benchmarking variant: run the whole kernel `repeat` times on
            # device so wall-clock timing can amortize the dispatch overhead
            loop_cm = tc.For_i(0, repeat, 1)
            loop_cm.__enter__()
        with (
            tc.tile_pool(name="persist", bufs=1) as pp,
            tc.tile_pool(name="xt_pool", bufs=3) as xp,
            tc.tile_pool(name="wka", bufs=3) as wka,
            tc.tile_pool(name="u_pool", bufs=6) as up,
            tc.tile_pool(name="tree_pool", bufs=1) as tp,
            tc.tile_pool(name="wkb", bufs=3) as wkb,
            # po gets dedicated PSUM banks for the whole kernel: if its banks
            # were reused from phase-A pools, the accumulation-start matmul
            # would carry a bank-WAR wait on top of its RAW wait (2 > limit).
            tc.tile_pool(name="poB", bufs=2, space="PSUM") as poB,
            # one dedicated bank shared (sequentially) by the absorber scratch
            # and the per-q-block dps/bps epilogue tiles, for the same reason
            tc.tile_pool(name="misc", bufs=1, space="PSUM") as mpool,
        ):
            # --- constants ---
            w_sb = {}
            for nm, w in (("q", wq), ("k", wk), ("v", wv)):
                t = pp.tile([P, DC * D_QK], MM_DT, tag=f"w{nm}", name=f"w{nm}_sb")
                nc.sync.dma_start(
                    out=t.rearrange("p (c n) -> p c n", n=D_QK),
                    in_=w.rearrange("(c p) n -> p c n", p=P),
                )
                w_sb[nm] = t
            b_sb = {}
            for nm, b in (("q", bq), ("k", bk), ("v", bv)):
                t = pp.tile([P, 1], F32, tag=f"b{nm}", name=f"b{nm}_sb")
                nc.sync.dma_start(out=t, in_=b.unsqueeze(1))
                b_sb[nm] = t
            ident = pp.tile([P, P], MM_DT, tag="ident")
            make_identity(nc, ident)
            ones_col = pp.tile([P, 1], F32, tag="ones_col")  # lhsT for partition-sum
            nc.gpsimd.memset(ones_col, 1.0)
            ones_col_b = pp.tile([P, 1], MM_DT, tag="ones_col_b")
            nc.vector.tensor_copy(out=ones_col_b, in_=ones_col)
            ones_row = pp.tile([1, P], F32, tag="ones_row")  # lhsT for bcast
            nc.gpsimd.memset(ones_row, 1.0)
            ones_row_r = pp.tile([1, P], F32R, tag="ones_row_r")
            nc.vector.tensor_copy(out=ones_row_r, in_=ones_row)

            kT = pp.tile([P, S], MM_DT, tag="kT")
            vn = pp.tile([P, S], MM_DT, tag="vn")  # natural-v chunks side by side
            qT = pp.tile([P, SQ], MM_DT, tag="qT")

            # Wait-absorbers: a matmul lowers to an LDWEIGHTS+MATMUL pair whose
            # LW struct can encode only ONE sync wait. Tile happily attaches
            # 2+ (e.g. weight-DMA lane + xt-DMA lane), which walrus rejects
            # ("Too many sync wait commands"). Tiny PE matmuls reading each
            # DMA'd/POOL-produced tile make the PE observe those semaphores
            # first, so real matmuls need at most one wait.
            babs = wka.tile([P, 1], F32, tag="babs")

            def pe_absorb_into(scr_ap, ap):
                a = ap[:, 0:1]
                if a.dtype not in (F32, BF16):
                    a = a.bitcast(F32)
                nc.tensor.matmul(scr_ap, a, a, start=True, stop=True)

            # --- phase A: projections ---
            with (
                tc.tile_pool(name="psA", bufs=3, space="PSUM") as psA,
                tc.tile_pool(name="tpsA", bufs=2, space="PSUM") as tpsA,
            ):
                scr = mpool.tile([1, 1], F32, tag="misc")

                def pe_absorb(ap):
                    pe_absorb_into(scr, ap)
                for nm in ("q", "k", "v"):
                    pe_absorb(w_sb[nm])  # weight DMA lanes
                    nc.scalar.copy(out=babs, in_=b_sb[nm])  # bias DMA lanes (ACT)
                # POOL-written tiles (each may be the last POOL tick after
                # scheduling, so absorb every one)
                pe_absorb(ident)
                pe_absorb(ones_col)
                pe_absorb(ones_col_b)
                pe_absorb(ones_row[0:1, 0:1].broadcast_to([1, 1]))
                pe_absorb(ones_row_r[0:1, 0:1].broadcast_to([1, 1]))

                for n in range(NB):
                    xt = xp.tile([P, DC * 512], MM_DT, tag="xt")
                    xt3 = xt.rearrange("p (c s) -> p c s", s=512)
                    xT3 = xT[:, ts(n, 512)].rearrange("(c p) s -> p c s", p=P)
                    hc = DC // 2
                    nc.sync.dma_start(out=xt3[:, :hc], in_=xT3[:, :hc])
                    nc.sync.dma_start(out=xt3[:, hc:], in_=xT3[:, hc:])
                    pe_absorb(xt)  # xt DMA lane
                    # kT block
                    kps = psA.tile([P, 512], F32, tag="ps")
                    for c in range(DC):
                        nc.tensor.matmul(
                            kps,
                            w_sb["k"][:, ts(c, D_QK)],
                            xt[:, ts(c, 512)],
                            start=(c == 0),
                            stop=(c == DC - 1),
                        )
                    nc.vector.tensor_scalar_add(
                        kT[:, ts(n, 512)], kps, b_sb["k"]
                    )
                    # v block: project to vT then PE-transpose to natural chunks
                    vps = psA.tile([P, 512], F32, tag="ps")
                    for c in range(DC):
                        nc.tensor.matmul(
                            vps,
                            w_sb["v"][:, ts(c, D_QK)],
                            xt[:, ts(c, 512)],
                            start=(c == 0),
                            stop=(c == DC - 1),
                        )
                    vt_tmp = wka.tile([P, 512], MM_DT, tag="vt_tmp")
                    nc.vector.tensor_scalar_add(vt_tmp, vps, b_sb["v"])
                    for j in range(4):
                        tps = tpsA.tile([P, P], MM_DT, tag="tps")
                        nc.tensor.transpose(tps, vt_tmp[:, ts(j, P)], ident)
                        nc.scalar.copy(out=vn[:, ts(4 * n + j, P)], in_=tps)
                    # qT block (first SQ columns only)
                    if n < QNB:
                        qps = psA.tile([P, 512], F32, tag="ps")
                        for c in range(DC):
                            nc.tensor.matmul(
                                qps,
                                w_sb["q"][:, ts(c, D_QK)],
                                xt[:, ts(c, 512)],
                                start=(c == 0),
                                stop=(c == DC - 1),
                            )
                        nc.vector.tensor_scalar_add(
                            qT[:, ts(n, 512)], qps, b_sb["q"]
                        )

            # --- phase B: attention ---
            with tc.tile_pool(name="psB", bufs=2, space="PSUM") as psB:
                for qb in range(QNB):
                    po = poB.tile([P, 512], F32, tag="po")
                    dps = mpool.tile([1, 512], F32, tag="misc", name=f"dps_{qb}")
                    us: dict[int, object] = {}
                    # denominator: sequential bf16 accumulation on DVE (2x
                    # mode, ~0.6us per [128,1024] add, hidden under the ~1us
                    # exp cadence).  Sequential (not a tree) so the final
                    # value is ready one add after the last exp -- the
                    # last-q-block epilogue tail is the whole-kernel tail.
                    dacc2 = tp.tile(
                        [P, 1024], MM_DT, tag="dacc2", name=f"dacc2_{qb}"
                    )
                    # Software-pipelined over k-chunk PAIRS: two scores
                    # matmuls land in the two banks of one [128,1024] PSUM
                    # tile, a single wide exp (ACT fixed cost ~185ns/op;
                    # ACT is the phase-B limiter) produces u bf16, then two
                    # PV matmuls.  Denominator: binary tree of bf16 DVE adds
                    # over the 16 u tiles, interleaved into the pipeline.
                    for mp in range(KP + 1):
                        if mp < KP:
                            sps = psB.tile([P, 1024], F32, tag="ps")
                            for h in range(2):
                                nc.tensor.matmul(
                                    sps[:, ts(h, 512)],
                                    kT[:, ts(2 * mp + h, P)],
                                    qT[:, ts(qb, 512)],
                                    start=True,
                                    stop=True,
                                )
                            u = up.tile([P, 1024], MM_DT, tag="u")
                            nc.scalar.activation(u, sps, AF.Exp, scale=SM_SCALE)
                            us[mp] = u
                            if mp == 0:
                                nc.vector.tensor_copy(out=dacc2, in_=u)
                            else:
                                nc.vector.tensor_add(
                                    out=dacc2, in0=dacc2, in1=u
                                )
                        if mp > 0:
                            u_prev = us.pop(mp - 1)
                            for h in range(2):
                                mm = 2 * (mp - 1) + h
                                nc.tensor.matmul(
                                    po,
                                    vn[:, ts(mm, P)],
                                    u_prev[:, ts(h, 512)],
                                    start=(mm == 0),
                                    stop=(mm == KC - 1),
                                )
                    dacc = tp.tile([P, 512], MM_DT, tag="dacc", name=f"dacc_{qb}")
                    nc.vector.tensor_add(
                        out=dacc, in0=dacc2[:, ts(0, 512)], in1=dacc2[:, ts(1, 512)]
                    )
                    nc.tensor.matmul(dps, ones_col_b, dacc, start=True, stop=True)
                    rec = wkb.tile([1, 512], F32R, tag="rec")
                    nc.vector.reciprocal(out=rec, in_=dps)
                    bps = mpool.tile([P, 512], F32, tag="misc", name=f"bps_{qb}")
                    nc.tensor.matmul(bps, ones_row_r, rec, start=True, stop=True)
                    # evacuate po and bps on DVE (ACT is the phase-B
                    # bottleneck; GPSIMD cannot access PSUM); absorb each so
                    # the next user of the PSUM bank carries only its RAW
                    # wait.  The absorber scratch is a FRESH tile on the misc
                    # ring (writing the phase-A scr tile here would clobber
                    # dps/bps, which share its buffer).
                    sab = mpool.tile([1, 1], F32, tag="misc", name=f"sab_{qb}")
                    bsb = wkb.tile([P, 512], F32, tag="bsb")
                    nc.vector.tensor_copy(out=bsb, in_=bps)
                    pe_absorb_into(sab, bsb)
                    poc = wkb.tile([P, 512], F32, tag="poc")
                    nc.vector.tensor_copy(out=poc, in_=po)
                    pe_absorb_into(sab, poc)
                    fin = wkb.tile([P, 512], F32, tag="fin")
                    nc.vector.tensor_mul(out=fin, in0=poc, in1=bsb)
                    nc.sync.dma_start(out=outT[:, ts(qb, 512)], in_=fin)

        if repeat > 1:
            loop_cm.__exit__(None, None, None)

    return nc


_NC_CACHE: dict = {}


def _get_nc(S: int = S_FULL, SQ: int = S_FULL // 2, repeat: int = 1):
    key = (S, SQ, repeat)
    if key not in _NC_CACHE:
        nc = bacc.Bacc("TRN2", debug=False)
        build_attention(nc, S, SQ, repeat)
        nc.compile()  # splits multi-waits into event semaphores (HW limit)
        _NC_CACHE[key] = nc
    return _NC_CACHE[key]


def _bf16(a):
    import ml_dtypes

    return np.ascontiguousarray(np.asarray(a, dtype=np.float32).astype(ml_dtypes.bfloat16))


def make_in_maps(x, Wq, bq, Wk, bk, Wv, bv):
    """Per-core input dicts. Core c = (batch c//2, query-half c%2)."""
    x = np.asarray(x, dtype=np.float32)
    common = {
        "Wq": _bf16(Wq),
        "bq": np.ascontiguousarray(bq, dtype=np.float32),
        "Wk": _bf16(Wk),
        "bk": np.ascontiguousarray(bk, dtype=np.float32),
        "Wv": _bf16(Wv),
        "bv": np.ascontiguousarray(bv, dtype=np.float32),
    }
    in_maps = []
    for c in range(N_CORES):
        b, h = divmod(c, 2)
        xb = x[b]  # [S, D]
        half = S_FULL // 2
        if h == 0:
            perm = xb
        else:
            perm = np.concatenate([xb[half:], xb[:half]], axis=0)
        in_maps.append({"xT": _bf16(perm.T), **common})
    return in_maps


def assemble_output(results):
    """results: list of 8 per-core dicts with 'outT' [128, 2048]."""
    half = S_FULL // 2
    out = np.empty((B, S_FULL, D_QK), dtype=np.float32)
    for c in range(N_CORES):
        b, h = divmod(c, 2)
        out[b, h * half : (h + 1) * half, :] = np.asarray(
            results[c]["outT"], dtype=np.float32
        ).T
    return out


def kernel(x, Wq, bq, Wk, bk, Wv, bv):
    from concourse.bass_utils import run_bass_kernel_spmd

    nc = _get_nc()
    in_maps = make_in_maps(x, Wq, bq, Wk, bk, Wv, bv)
    res = run_bass_kernel_spmd(nc, in_maps, list(range(N_CORES)))
    return assemble_output(res.results)


# revision 39
# speedup vs baseline: 1.3255x; 1.0975x over previous
"""Multi-head(1) attention kernel for Trainium2, 8 NeuronCores SPMD.

Problem: x[4,4096,1024] @ {Wq,Wk,Wv}[1024,128] -> q,k,v; softmax(q k^T/sqrt(128)) v.

Sharding: core c -> (batch b = c//2, query-half h = c%2).
Each core receives xT = x[b].T (d_model on rows) with the 4096 columns permuted
so that "my" 2048 query rows come first. The core computes kT/v for all 4096
keys (key order is irrelevant under softmax), qT for its first 2048 columns,
and emits outT [128, 2048] = (attention output for its query rows)^T.

v3 (fused): all matmul operands bfloat16 (host converts x and W; halves HBM
traffic, keeps PE at 1 cyc/row, unlocks DVE 2x adds).  The projection sweep
and the attention of the first NFQ=3 query blocks are FUSED into one pass:
query block qq processes key block (n - qq) during round n (the lag keeps
every operand one round old, so nothing in the attention stream waits on
same-round projections).  Projection matmuls for round n+1 are emitted as
fine-grained "background ops" interleaved between attention chunk-iters so
the in-order PE stream always has non-dependent work while ACT exps run.
The 4th query block runs as a short pass 2 (paired [128,1024] exps) that is
ACT-bound; its PE work fits in the tail.

Engine budget (per core, 2.4GHz PE / 1.2GHz ACT / 0.96GHz DVE):
  pass 1: PE 187k cyc (proj 86k + scores/PV 98k + epilogues) ~78us;
          ACT 96 exps x 612ns + 32 transpose-evac copies ~68us; DVE ~55us.
  pass 2: ACT-bound ~17.5us.  Denominators: per-key-block folds + sequential
  accumulate, all bf16 on DVE; one ones-matmul partition sum per q-block.

PSUM (16KB/partition): po_0..2 (6KB) + proj ring (4KB) + scores ring (4KB)
+ transpose ring (0.5KB); epilogue dps/bps and absorber scratch ride the
scores ring.  Pass 2 swaps the pass-1 rings for a [128,1024] pair ring.

Absorber matmuls (tiny PE reads of DMA'd/POOL-produced tiles) keep real
matmuls at <=1 sync wait (LDWEIGHTS can encode only one).
"""

import math

import numpy as np

import concourse.bacc as bacc
import concourse.bass as bass
import concourse.mybir as mybir
from concourse.bass import ts
from concourse.tile import TileContext

P = 128
D_MODEL = 1024
D_QK = 128
B = 4
S_FULL = 4096
N_CORES = 8

F32 = mybir.dt.float32
F32R = mybir.dt.float32r
BF16 = mybir.dt.bfloat16
AF = mybir.ActivationFunctionType

SM_SCALE = 1.0 / math.sqrt(D_QK)

MM_DT = BF16


def build_attention(nc: bass.Bass, S: int = S_FULL, SQ: int = S_FULL // 2, repeat: int = 1):
    """Emit the SPMD single-core program. S = #keys, SQ = #queries."""
    assert S % 512 == 0 and SQ % 512 == 0 and D_MODEL % P == 0
    DC = D_MODEL // P  # 8 d_model chunks
    NB = S // 512  # xT column blocks
    QNB = SQ // 512  # of which query blocks
    KC = S // P  # 32 k chunks
    KP = KC // 2  # k-chunk pairs (pass 2)
    NFQ = min(3, QNB)  # query blocks fused into the projection sweep

    xT = nc.dram_tensor("xT", [D_MODEL, S], MM_DT, kind="ExternalInput").ap()
    wq = nc.dram_tensor("Wq", [D_MODEL, D_QK], MM_DT, kind="ExternalInput").ap()
    bq = nc.dram_tensor("bq", [D_QK], F32, kind="ExternalInput").ap()
    wk = nc.dram_tensor("Wk", [D_MODEL, D_QK], MM_DT, kind="ExternalInput").ap()
    bk = nc.dram_tensor("bk", [D_QK], F32, kind="ExternalInput").ap()
    wv = nc.dram_tensor("Wv", [D_MODEL, D_QK], MM_DT, kind="ExternalInput").ap()
    bv = nc.dram_tensor("bv", [D_QK], F32, kind="ExternalInput").ap()
    outT = nc.dram_tensor("outT", [D_QK, SQ], F32, kind="ExternalOutput").ap()

    with TileContext(nc) as tc:
        lp = nc.allow_low_precision(reason="bf16 matmuls + bf16 denominator accum")
        lp.__enter__()
        if repeat > 1:
            loop_cm = tc.For_i(0, repeat, 1)
            loop_cm.__enter__()
        with (
            tc.tile_pool(name="persist", bufs=1) as pp,
            tc.tile_pool(name="xt_pool", bufs=3) as xp,
            tc.tile_pool(name="wka", bufs=3) as wka,
            tc.tile_pool(name="u_pool", bufs=6) as up,
            tc.tile_pool(name="fold_pool", bufs=2) as fp,
            tc.tile_pool(name="acc_pool", bufs=1) as accp,
            tc.tile_pool(name="wkb", bufs=3) as wkb,
            tc.tile_pool(name="poB", bufs=1, space="PSUM") as poB,
        ):
            # --- constants ---
            w_sb = {}
            for nm, w in (("q", wq), ("k", wk), ("v", wv)):
                t = pp.tile([P, DC * D_QK], MM_DT, tag=f"w{nm}", name=f"w{nm}_sb")
                nc.sync.dma_start(
                    out=t.rearrange("p (c n) -> p c n", n=D_QK),
                    in_=w.rearrange("(c p) n -> p c n", p=P),
                )
                w_sb[nm] = t
            b_sb = {}
            for nm, b in (("q", bq), ("k", bk), ("v", bv)):
                t = pp.tile([P, 1], F32, tag=f"b{nm}", name=f"b{nm}_sb")
                nc.sync.dma_start(out=t, in_=b.unsqueeze(1))
                b_sb[nm] = t
            ones_col = pp.tile([P, 1], F32, tag="ones_col")
            nc.gpsimd.memset(ones_col, 1.0)
            ones_col_b = pp.tile([P, 1], MM_DT, tag="ones_col_b")
            nc.vector.tensor_copy(out=ones_col_b, in_=ones_col)
            ones_row = pp.tile([1, P], F32, tag="ones_row")
            nc.gpsimd.memset(ones_row, 1.0)
            ones_row_r = pp.tile([1, P], F32R, tag="ones_row_r")
            nc.vector.tensor_copy(out=ones_row_r, in_=ones_row)

            kT = pp.tile([P, S], MM_DT, tag="kT")
            vn = pp.tile([P, S], MM_DT, tag="vn")
            qT = pp.tile([P, SQ], MM_DT, tag="qT")
            vb = pp.tile([P, P], F32, tag="vb")  # bv broadcast across partitions
            bv_row = pp.tile([1, P], F32, tag="bv_row")
            nc.sync.dma_start(out=bv_row, in_=bv.unsqueeze(0))

            babs = wka.tile([P, 1], F32, tag="babs")

            # --- pass 1: fused projection sweep + attention of qq < NFQ ---
            with (
                tc.tile_pool(name="psA", bufs=2, space="PSUM") as psA,
                tc.tile_pool(name="sps1", bufs=3, space="PSUM") as sp1,
            ):
                scrn = [0]
                absorb_alloc = [
                    lambda name: psA.tile([1, 1], F32, tag="kqv", name=name)
                ]

                def pe_absorb(ap):
                    # tiny PE matmul reading `ap` into a fresh ring slot so
                    # later real matmuls carry at most one sync wait.
                    scrn[0] += 1
                    s = absorb_alloc[0](f"scr_{scrn[0]}")
                    a = ap[:, 0:1]
                    if a.dtype not in (F32, BF16):
                        a = a.bitcast(F32)
                    nc.tensor.matmul(s, a, a, start=True, stop=True)

                for nm in ("q", "k", "v"):
                    pe_absorb(w_sb[nm])
                    nc.scalar.copy(out=babs, in_=b_sb[nm])
                pe_absorb(ones_col)
                pe_absorb(ones_col_b)
                pe_absorb(ones_row[0:1, 0:1].broadcast_to([1, 1]))
                pe_absorb(ones_row_r[0:1, 0:1].broadcast_to([1, 1]))
                # vb = ones ^T bv  (bias along the free dim of natural-v)
                vbps = psA.tile([P, P], F32, tag="kqv", name="vbps")
                nc.tensor.matmul(vbps, ones_row, bv_row, start=True, stop=True)
                nc.vector.tensor_copy(out=vb, in_=vbps)

                xts: dict[int, object] = {}

                def emit_xt_dma(n, parts=2):
                    # parts spread over two idle DGE queues (SP + Pool): the
                    # per-chunk consumers each wait on exactly one queue sem,
                    # and the queues generate/transfer in parallel.
                    xt = xp.tile([P, DC * 512], MM_DT, tag="xt", name=f"xt_{n}")
                    xt3 = xt.rearrange("p (c s) -> p c s", s=512)
                    xT3 = xT[:, ts(n, 512)].rearrange("(c p) s -> p c s", p=P)
                    w = DC // parts
                    for i in range(parts):
                        nc.sync.dma_start(
                            out=xt3[:, i * w : (i + 1) * w],
                            in_=xT3[:, i * w : (i + 1) * w],
                        )
                    xts[n] = xt

                def proj_ops(n):
                    """Background closures emitting projections for block n
                    (consumed by attention one round later)."""
                    ops = []
                    if n + 2 < NB:
                        # prefetch two blocks ahead (xt ring is 3 deep; this
                        # block's closures run one round before consumption)
                        ops.append(lambda: emit_xt_dma(n + 2))
                    ops.append(lambda: pe_absorb(xts[n]))
                    state: dict = {}

                    def mk_proj(nm, c, first, last, dest):
                        def _op():
                            if first:
                                state[nm] = psA.tile(
                                    [P, 512], F32, tag="kqv", name=f"{nm}ps_{n}"
                                )
                            nc.tensor.matmul(
                                state[nm],
                                w_sb[nm][:, ts(c, D_QK)],
                                xts[n][:, ts(c, 512)],
                                start=first,
                                stop=last,
                            )
                            if last and dest is not None:
                                nc.vector.tensor_scalar_add(
                                    dest, state[nm], b_sb[nm]
                                )

                        return _op

                    for c in range(DC):
                        ops.append(
                            mk_proj("k", c, c == 0, c == DC - 1, kT[:, ts(n, 512)])
                        )
                    if n < QNB:
                        for c in range(DC):
                            ops.append(
                                mk_proj("q", c, c == 0, c == DC - 1, qT[:, ts(n, 512)])
                            )
                    # natural-layout v: out[key, dv] accumulated over d_model
                    # chunks with the x slice as the stationary operand; no
                    # transposes, no extra PSUM pool.  128-row matmuls, two
                    # per background op.
                    xt3v = None

                    def mk_vproj(j, cc):
                        def _op():
                            nonlocal xt3v
                            key = f"v{j}"
                            if cc == 0:
                                state[key] = psA.tile(
                                    [P, P], F32, tag="kqv", name=f"vps_{n}_{j}"
                                )
                            vps = state[key]
                            for c in (cc, cc + 1):
                                nc.tensor.matmul(
                                    vps,
                                    xts[n][:, c * 512 + j * P : c * 512 + (j + 1) * P],
                                    w_sb["v"][:, ts(c, D_QK)],
                                    start=(c == 0),
                                    stop=(c == DC - 1),
                                )
                            if cc + 2 == DC:
                                nc.vector.tensor_add(
                                    out=vn[:, ts(4 * n + j, P)], in0=vps, in1=vb
                                )

                        return _op

                    for j in range(4):
                        for cc in range(0, DC, 2):
                            ops.append(mk_vproj(j, cc))
                    return ops

                accs: dict[int, object] = {}
                pend_pv: list = []
                PVDEPTH = 2

                def emit_pv(qq, ck, u1):
                    nc.tensor.matmul(
                        pos[qq],
                        vn[:, ts(ck, P)],
                        u1,
                        start=(ck == 0),
                        stop=(ck == KC - 1),
                    )

                pos = {
                    qq: poB.tile([P, 512], F32, tag=f"po_{qq}", name=f"po_{qq}")
                    for qq in range(NFQ)
                }

                def epilogue_ops(qq, po, ring, halves):
                    """Closures: softmax denominator -> normalize -> DMA out.
                    `ring` supplies PSUM tiles (pass-1: sps1, pass-2: psB);
                    `halves` lazily yields the two [128,512] bf16 partial-sum
                    tiles whose partition+pair sum is the denominator."""
                    st: dict = {}

                    def e_abs():
                        pe_absorb(halves()[0])

                    def e_dps():
                        st["dps"] = ring([1, 512], F32, f"dps_{qq}")
                        h = halves()
                        nc.tensor.matmul(
                            st["dps"], ones_col_b, h[0], start=True, stop=False
                        )
                        nc.tensor.matmul(
                            st["dps"], ones_col_b, h[1], start=False, stop=True
                        )

                    def e_rec():
                        # po evacuation overlaps the PE's dps/bps matmuls
                        st["poc"] = wkb.tile([P, 512], F32, tag="poc", name=f"poc_{qq}")
                        nc.vector.tensor_copy(out=st["poc"], in_=po)
                        st["rec"] = wkb.tile([1, 512], F32R, tag="rec", name=f"rec_{qq}")
                        nc.vector.reciprocal(out=st["rec"], in_=st["dps"])

                    def e_bps():
                        st["bps"] = ring([P, 512], F32, f"bps_{qq}")
                        nc.tensor.matmul(
                            st["bps"], ones_row_r, st["rec"], start=True, stop=True
                        )

                    def e_fin():
                        # bps is read straight out of PSUM; the one absorber
                        # transitively clears every epilogue WAR (fin waits
                        # rec/poc which wait dps/po)
                        fin = wkb.tile([P, 512], F32, tag="fin", name=f"fin_{qq}")
                        nc.vector.tensor_mul(out=fin, in0=st["poc"], in1=st["bps"])
                        nc.sync.dma_start(out=outT[:, ts(qq, 512)], in_=fin)
                        pe_absorb(fin)

                    return [e_abs, e_dps, e_rec, e_bps, e_fin]

                def sps_ring(shape, dtype, name):
                    return sp1.tile(shape, dtype, tag="ps", name=name)

                # late-bound ring for epilogues that may spill into pass 2
                # (after the pass-1 PSUM pools close)
                ring_cell = [sps_ring]

                def late_ring(shape, dtype, name):
                    return ring_cell[0](shape, dtype, name)

                # prologue: first two x blocks + projections for block 0
                # (proj_ops(n) prefetches x block n+2)
                emit_xt_dma(0, parts=4)
                if NB > 1:
                    emit_xt_dma(1, parts=4)
                for op in proj_ops(0):
                    op()

                n_rounds = NB + NFQ - 1
                for n in range(n_rounds):
                    bg = []
                    if n + 1 < NB:
                        bg += proj_ops(n + 1)
                    for qq in range(NFQ):
                        if n == NB + qq:
                            bg += epilogue_ops(
                                qq, pos[qq], sps_ring, lambda qq=qq: accs[qq]
                            )
                    items = [
                        (qq, n - qq)
                        for qq in range(NFQ)
                        if 0 <= n - qq < NB
                    ]
                    n_iters = 4 * len(items)
                    it = 0
                    for qq, blk in items:
                        ust: dict = {}
                        for j in range(4):
                            # interleave background (projection/epilogue) ops
                            want = (len(bg) * (it + 1)) // n_iters
                            done = (len(bg) * it) // n_iters
                            for _ in range(want - done):
                                bg_op = bg[done]
                                done += 1
                                bg_op()
                            it += 1
                            ck = 4 * blk + j
                            sp = sp1.tile(
                                [P, 512], F32, tag="ps", name=f"sp_{qq}_{ck}"
                            )
                            nc.tensor.matmul(
                                sp,
                                kT[:, ts(ck, P)],
                                qT[:, ts(qq, 512)],
                                start=True,
                                stop=True,
                            )
                            u1 = up.tile(
                                [P, 512], MM_DT, tag="u1", name=f"u_{qq}_{ck}"
                            )
                            nc.scalar.activation(u1, sp, AF.Exp, scale=SM_SCALE)
                            ust[j] = u1
                            if j == 1:
                                fa = fp.tile([P, 512], MM_DT, tag="fA", name=f"fA_{qq}_{blk}")
                                nc.vector.tensor_add(out=fa, in0=ust[0], in1=ust[1])
                                ust["fa"] = fa
                            if j == 3:
                                fb = fp.tile([P, 512], MM_DT, tag="fB", name=f"fB_{qq}_{blk}")
                                nc.vector.tensor_add(out=fb, in0=ust[2], in1=ust[3])
                                fc = fp.tile([P, 512], MM_DT, tag="fC", name=f"fC_{qq}_{blk}")
                                nc.vector.tensor_add(out=fc, in0=ust["fa"], in1=fb)
                                if blk == 0:
                                    acc = accp.tile(
                                        [P, 512], MM_DT, tag=f"acc_{qq}",
                                        name=f"acc_{qq}",
                                    )
                                    accs[qq] = (acc,)
                                    nc.vector.tensor_copy(out=acc, in_=fc)
                                elif blk == NB - 1:
                                    # final add goes to a second tile so the
                                    # epilogue's dacc fold has two operands
                                    accs[qq] = (accs[qq][0], fc)
                                else:
                                    nc.vector.tensor_add(
                                        out=accs[qq][0], in0=accs[qq][0], in1=fc
                                    )
                            pend_pv.append((qq, ck, u1))
                            if len(pend_pv) > PVDEPTH:
                                emit_pv(*pend_pv.pop(0))
                    if n_iters == 0:  # defensive: rounds with no att items
                        for bg_op in bg:
                            bg_op()
                while pend_pv:
                    emit_pv(*pend_pv.pop(0))
                # epilogues not emitted inside rounds spill into pass 2 (or,
                # if there is no pass 2, run here)
                spill: list = []
                for qq in range(NFQ):
                    if NB + qq > n_rounds - 1:
                        spill += epilogue_ops(
                            qq, pos[qq], late_ring, lambda qq=qq: accs[qq]
                        )
                if QNB == NFQ:
                    for op in spill:
                        op()
                    spill = []

            # --- pass 2: remaining query blocks, paired-exp pipeline ---
            if QNB > NFQ:
                with tc.tile_pool(name="psB", bufs=2, space="PSUM") as psB:

                    def psb_ring(shape, dtype, name):
                        return psB.tile(shape, dtype, tag="ps", name=name)

                    absorb_alloc[0] = lambda name: psb_ring([1, 1], F32, name)
                    ring_cell[0] = psb_ring

                    for qb in range(NFQ, QNB):
                        bg2 = spill
                        spill = []
                        po = poB.tile(
                            [P, 512], F32, tag=f"po_{qb % NFQ}", name=f"po2_{qb}"
                        )
                        us: dict[int, object] = {}
                        dacc2 = accp.tile(
                            [P, 1024], MM_DT, tag="dacc2", name=f"dacc2_{qb}"
                        )
                        for mp in range(KP + 1):
                            if bg2:
                                bg2.pop(0)()
                            if mp < KP:
                                sps = psB.tile(
                                    [P, 1024], F32, tag="ps", name=f"sps_{qb}_{mp}"
                                )
                                for h in range(2):
                                    nc.tensor.matmul(
                                        sps[:, ts(h, 512)],
                                        kT[:, ts(2 * mp + h, P)],
                                        qT[:, ts(qb, 512)],
                                        start=True,
                                        stop=True,
                                    )
                                u = up.tile([P, 1024], MM_DT, tag="u", name=f"u2_{qb}_{mp}")
                                nc.scalar.activation(u, sps, AF.Exp, scale=SM_SCALE)
                                us[mp] = u
                                if mp == 0:
                                    nc.vector.tensor_copy(out=dacc2, in_=u)
                                else:
                                    nc.vector.tensor_add(out=dacc2, in0=dacc2, in1=u)
                            if mp > 0:
                                u_prev = us.pop(mp - 1)
                                for h in range(2):
                                    mm = 2 * (mp - 1) + h
                                    nc.tensor.matmul(
                                        po,
                                        vn[:, ts(mm, P)],
                                        u_prev[:, ts(h, 512)],
                                        start=(mm == 0),
                                        stop=(mm == KC - 1),
                                    )
                        for op in epilogue_ops(
                            qb, po, psb_ring,
                            lambda d=dacc2: (d[:, ts(0, 512)], d[:, ts(1, 512)]),
                        ):
                            op()

        if repeat > 1:
            loop_cm.__exit__(None, None, None)

    return nc


_NC_CACHE: dict = {}


def _get_nc(S: int = S_FULL, SQ: int = S_FULL // 2, repeat: int = 1):
    key = (S, SQ, repeat)
    if key not in _NC_CACHE:
        nc = bacc.Bacc("TRN2", debug=False)
        build_attention(nc, S, SQ, repeat)
        nc.compile()  # splits multi-waits into event semaphores (HW limit)
        _NC_CACHE[key] = nc
    return _NC_CACHE[key]


def _bf16(a):
    import ml_dtypes

    return np.ascontiguousarray(np.asarray(a, dtype=np.float32).astype(ml_dtypes.bfloat16))


def make_in_maps(x, Wq, bq, Wk, bk, Wv, bv):
    """Per-core input dicts. Core c = (batch c//2, query-half c%2)."""
    x = np.asarray(x, dtype=np.float32)
    common = {
        "Wq": _bf16(Wq),
        "bq": np.ascontiguousarray(bq, dtype=np.float32),
        "Wk": _bf16(Wk),
        "bk": np.ascontiguousarray(bk, dtype=np.float32),
        "Wv": _bf16(Wv),
        "bv": np.ascontiguousarray(bv, dtype=np.float32),
    }
    in_maps = []
    for c in range(N_CORES):
        b, h = divmod(c, 2)
        xb = x[b]  # [S, D]
        half = S_FULL // 2
        if h == 0:
            perm = xb
        else:
            perm = np.concatenate([xb[half:], xb[:half]], axis=0)
        in_maps.append({"xT": _bf16(perm.T), **common})
    return in_maps


def assemble_output(results):
    """results: list of 8 per-core dicts with 'outT' [128, 2048]."""
    half = S_FULL // 2
    out = np.empty((B, S_FULL, D_QK), dtype=np.float32)
    for c in range(N_CORES):
        b, h = divmod(c, 2)
        out[b, h * half : (h + 1) * half, :] = np.asarray(
            results[c]["outT"], dtype=np.float32
        ).T
    return out


def kernel(x, Wq, bq, Wk, bk, Wv, bv):
    from concourse.bass_utils import run_bass_kernel_spmd

    nc = _get_nc()
    in_maps = make_in_maps(x, Wq, bq, Wk, bk, Wv, bv)
    res = run_bass_kernel_spmd(nc, in_maps, list(range(N_CORES)))
    return assemble_output(res.results)


# revision 41
# speedup vs baseline: 1.3525x; 1.0204x over previous
"""Multi-head(1) attention kernel for Trainium2, 8 NeuronCores SPMD.

Problem: x[4,4096,1024] @ {Wq,Wk,Wv}[1024,128] -> q,k,v; softmax(q k^T/sqrt(128)) v.

Sharding: core c -> (batch b = c//2, query-half h = c%2).
Each core receives xT = x[b].T (d_model on rows) with the 4096 columns permuted
so that "my" 2048 query rows come first. The core computes kT/v for all 4096
keys (key order is irrelevant under softmax), qT for its first 2048 columns,
and emits outT [128, 2048] = (attention output for its query rows)^T.

v3 (fused): all matmul operands bfloat16 (host converts x and W; halves HBM
traffic, keeps PE at 1 cyc/row, unlocks DVE 2x adds).  The projection sweep
and the attention of the first NFQ=3 query blocks are FUSED into one pass:
query block qq processes key block (n - qq) during round n (the lag keeps
every operand one round old, so nothing in the attention stream waits on
same-round projections).  Projection matmuls for round n+1 are emitted as
fine-grained "background ops" interleaved between attention chunk-iters so
the in-order PE stream always has non-dependent work while ACT exps run.
The 4th query block runs as a short pass 2 (paired [128,1024] exps) that is
ACT-bound; its PE work fits in the tail.

Engine budget (per core, 2.4GHz PE / 1.2GHz ACT / 0.96GHz DVE):
  pass 1: PE 187k cyc (proj 86k + scores/PV 98k + epilogues) ~78us;
          ACT 96 exps x 612ns + 32 transpose-evac copies ~68us; DVE ~55us.
  pass 2: ACT-bound ~17.5us.  Denominators: per-key-block folds + sequential
  accumulate, all bf16 on DVE; one ones-matmul partition sum per q-block.

PSUM (16KB/partition): po_0..2 (6KB) + proj ring (4KB) + scores ring (4KB)
+ transpose ring (0.5KB); epilogue dps/bps and absorber scratch ride the
scores ring.  Pass 2 swaps the pass-1 rings for a [128,1024] pair ring.

Absorber matmuls (tiny PE reads of DMA'd/POOL-produced tiles) keep real
matmuls at <=1 sync wait (LDWEIGHTS can encode only one).
"""

import math

import numpy as np

import concourse.bacc as bacc
import concourse.bass as bass
import concourse.mybir as mybir
from concourse.bass import ts
from concourse.tile import TileContext

P = 128
D_MODEL = 1024
D_QK = 128
B = 4
S_FULL = 4096
N_CORES = 8

F32 = mybir.dt.float32
F32R = mybir.dt.float32r
BF16 = mybir.dt.bfloat16
AF = mybir.ActivationFunctionType

SM_SCALE = 1.0 / math.sqrt(D_QK)

MM_DT = BF16


def build_attention(nc: bass.Bass, S: int = S_FULL, SQ: int = S_FULL // 2, repeat: int = 1):
    """Emit the SPMD single-core program. S = #keys, SQ = #queries."""
    assert S % 512 == 0 and SQ % 512 == 0 and D_MODEL % P == 0
    DC = D_MODEL // P  # 8 d_model chunks
    NB = S // 512  # xT column blocks
    QNB = SQ // 512  # of which query blocks
    KC = S // P  # 32 k chunks
    KP = KC // 2  # k-chunk pairs (pass 2)
    NFQ = min(3, QNB)  # query blocks fused into the projection sweep

    xT = nc.dram_tensor("xT", [D_MODEL, S], MM_DT, kind="ExternalInput").ap()
    wq = nc.dram_tensor("Wq", [D_MODEL, D_QK], MM_DT, kind="ExternalInput").ap()
    bq = nc.dram_tensor("bq", [D_QK], F32, kind="ExternalInput").ap()
    wk = nc.dram_tensor("Wk", [D_MODEL, D_QK], MM_DT, kind="ExternalInput").ap()
    bk = nc.dram_tensor("bk", [D_QK], F32, kind="ExternalInput").ap()
    wv = nc.dram_tensor("Wv", [D_MODEL, D_QK], MM_DT, kind="ExternalInput").ap()
    bv = nc.dram_tensor("bv", [D_QK], F32, kind="ExternalInput").ap()
    outT = nc.dram_tensor("outT", [D_QK, SQ], F32, kind="ExternalOutput").ap()

    with TileContext(nc) as tc:
        lp = nc.allow_low_precision(reason="bf16 matmuls + bf16 denominator accum")
        lp.__enter__()
        if repeat > 1:
            loop_cm = tc.For_i(0, repeat, 1)
            loop_cm.__enter__()
        with (
            tc.tile_pool(name="persist", bufs=1) as pp,
            tc.tile_pool(name="xt_pool", bufs=3) as xp,
            tc.tile_pool(name="wka", bufs=3) as wka,
            tc.tile_pool(name="u_pool", bufs=6) as up,
            tc.tile_pool(name="fold_pool", bufs=2) as fp,
            tc.tile_pool(name="acc_pool", bufs=1) as accp,
            tc.tile_pool(name="wkb", bufs=3) as wkb,
            tc.tile_pool(name="poB", bufs=1, space="PSUM") as poB,
        ):
            # --- constants ---
            w_sb = {}
            for nm, w in (("q", wq), ("k", wk), ("v", wv)):
                t = pp.tile([P, DC * D_QK], MM_DT, tag=f"w{nm}", name=f"w{nm}_sb")
                nc.sync.dma_start(
                    out=t.rearrange("p (c n) -> p c n", n=D_QK),
                    in_=w.rearrange("(c p) n -> p c n", p=P),
                )
                w_sb[nm] = t
            b_sb = {}
            for nm, b in (("q", bq), ("k", bk), ("v", bv)):
                t = pp.tile([P, 1], F32, tag=f"b{nm}", name=f"b{nm}_sb")
                nc.sync.dma_start(out=t, in_=b.unsqueeze(1))
                b_sb[nm] = t
            ones_col = pp.tile([P, 1], F32, tag="ones_col")
            nc.gpsimd.memset(ones_col, 1.0)
            ones_col_b = pp.tile([P, 1], MM_DT, tag="ones_col_b")
            nc.vector.tensor_copy(out=ones_col_b, in_=ones_col)
            ones_row = pp.tile([1, P], F32, tag="ones_row")
            nc.gpsimd.memset(ones_row, 1.0)
            ones_row_r = pp.tile([1, P], F32R, tag="ones_row_r")
            nc.vector.tensor_copy(out=ones_row_r, in_=ones_row)

            kT = pp.tile([P, S], MM_DT, tag="kT")
            vn = pp.tile([P, S], MM_DT, tag="vn")
            qT = pp.tile([P, SQ], MM_DT, tag="qT")
            vb = pp.tile([P, P], F32, tag="vb")  # bv broadcast across partitions
            bv_row = pp.tile([1, P], F32, tag="bv_row")
            nc.sync.dma_start(out=bv_row, in_=bv.unsqueeze(0))

            babs = wka.tile([P, 1], F32, tag="babs")

            # --- pass 1: fused projection sweep + attention of qq < NFQ ---
            with (
                tc.tile_pool(name="psA", bufs=2, space="PSUM") as psA,
                tc.tile_pool(name="sps1", bufs=3, space="PSUM") as sp1,
            ):
                scrn = [0]
                absorb_alloc = [
                    lambda name: psA.tile([1, 1], F32, tag="kqv", name=name)
                ]

                def pe_absorb(ap):
                    # tiny PE matmul reading `ap` into a fresh ring slot so
                    # later real matmuls carry at most one sync wait.
                    scrn[0] += 1
                    s = absorb_alloc[0](f"scr_{scrn[0]}")
                    a = ap[:, 0:1]
                    if a.dtype not in (F32, BF16):
                        a = a.bitcast(F32)
                    nc.tensor.matmul(s, a, a, start=True, stop=True)

                for nm in ("q", "k", "v"):
                    pe_absorb(w_sb[nm])
                    nc.scalar.copy(out=babs, in_=b_sb[nm])
                pe_absorb(ones_col)
                pe_absorb(ones_col_b)
                pe_absorb(ones_row[0:1, 0:1].broadcast_to([1, 1]))
                pe_absorb(ones_row_r[0:1, 0:1].broadcast_to([1, 1]))
                # vb = ones ^T bv  (bias along the free dim of natural-v)
                vbps = psA.tile([P, P], F32, tag="kqv", name="vbps")
                nc.tensor.matmul(vbps, ones_row, bv_row, start=True, stop=True)
                nc.vector.tensor_copy(out=vb, in_=vbps)

                xts: dict[int, object] = {}

                def emit_xt_dma(n, parts=4):
                    # parts spread over two idle DGE queues (SP + Pool): the
                    # per-chunk consumers each wait on exactly one queue sem,
                    # and the queues generate/transfer in parallel.
                    xt = xp.tile([P, DC * 512], MM_DT, tag="xt", name=f"xt_{n}")
                    xt3 = xt.rearrange("p (c s) -> p c s", s=512)
                    xT3 = xT[:, ts(n, 512)].rearrange("(c p) s -> p c s", p=P)
                    w = DC // parts
                    for i in range(parts):
                        nc.sync.dma_start(
                            out=xt3[:, i * w : (i + 1) * w],
                            in_=xT3[:, i * w : (i + 1) * w],
                        )
                    xts[n] = xt

                def proj_ops(n):
                    """Background closures emitting projections for block n
                    (consumed by attention one round later)."""
                    ops = []
                    if n + 2 < NB:
                        # prefetch two blocks ahead (xt ring is 3 deep; this
                        # block's closures run one round before consumption)
                        ops.append(lambda: emit_xt_dma(n + 2))
                    ops.append(lambda: pe_absorb(xts[n]))
                    state: dict = {}

                    def mk_proj(nm, c, first, last, dest):
                        def _op():
                            if first:
                                state[nm] = psA.tile(
                                    [P, 512], F32, tag="kqv", name=f"{nm}ps_{n}"
                                )
                            nc.tensor.matmul(
                                state[nm],
                                w_sb[nm][:, ts(c, D_QK)],
                                xts[n][:, ts(c, 512)],
                                start=first,
                                stop=last,
                            )
                            if last and dest is not None:
                                nc.vector.tensor_scalar_add(
                                    dest, state[nm], b_sb[nm]
                                )

                        return _op

                    for c in range(DC):
                        ops.append(
                            mk_proj("k", c, c == 0, c == DC - 1, kT[:, ts(n, 512)])
                        )
                    if n < QNB:
                        for c in range(DC):
                            ops.append(
                                mk_proj("q", c, c == 0, c == DC - 1, qT[:, ts(n, 512)])
                            )
                    # natural-layout v: out[key, dv] accumulated over d_model
                    # chunks with the x slice as the stationary operand; no
                    # transposes, no extra PSUM pool.  128-row matmuls, two
                    # per background op.
                    xt3v = None

                    def mk_vproj(j, cc):
                        def _op():
                            nonlocal xt3v
                            key = f"v{j}"
                            if cc == 0:
                                state[key] = psA.tile(
                                    [P, P], F32, tag="kqv", name=f"vps_{n}_{j}"
                                )
                            vps = state[key]
                            for c in (cc, cc + 1):
                                nc.tensor.matmul(
                                    vps,
                                    xts[n][:, c * 512 + j * P : c * 512 + (j + 1) * P],
                                    w_sb["v"][:, ts(c, D_QK)],
                                    start=(c == 0),
                                    stop=(c == DC - 1),
                                )
                            if cc + 2 == DC:
                                nc.vector.tensor_add(
                                    out=vn[:, ts(4 * n + j, P)], in0=vps, in1=vb
                                )

                        return _op

                    for j in range(4):
                        for cc in range(0, DC, 2):
                            ops.append(mk_vproj(j, cc))
                    return ops

                accs: dict[int, object] = {}
                pend_pv: list = []
                PVDEPTH = 3

                def emit_pv(qq, ck, u1):
                    nc.tensor.matmul(
                        pos[qq],
                        vn[:, ts(ck, P)],
                        u1,
                        start=(ck == 0),
                        stop=(ck == KC - 1),
                    )

                pos = {
                    qq: poB.tile([P, 512], F32, tag=f"po_{qq}", name=f"po_{qq}")
                    for qq in range(NFQ)
                }

                def epilogue_ops(qq, po, ring, halves):
                    """Closures: softmax denominator -> normalize -> DMA out.
                    `ring` supplies PSUM tiles (pass-1: sps1, pass-2: psB);
                    `halves` lazily yields the two [128,512] bf16 partial-sum
                    tiles whose partition+pair sum is the denominator."""
                    st: dict = {}

                    def e_abs():
                        pe_absorb(halves()[0])

                    def e_dps():
                        st["dps"] = ring([1, 512], F32, f"dps_{qq}")
                        h = halves()
                        nc.tensor.matmul(
                            st["dps"], ones_col_b, h[0], start=True, stop=False
                        )
                        nc.tensor.matmul(
                            st["dps"], ones_col_b, h[1], start=False, stop=True
                        )

                    def e_rec():
                        # po evacuation overlaps the PE's dps/bps matmuls
                        st["poc"] = wkb.tile([P, 512], F32, tag="poc", name=f"poc_{qq}")
                        nc.vector.tensor_copy(out=st["poc"], in_=po)
                        st["rec"] = wkb.tile([1, 512], F32R, tag="rec", name=f"rec_{qq}")
                        nc.vector.reciprocal(out=st["rec"], in_=st["dps"])

                    def e_bps():
                        st["bps"] = ring([P, 512], F32, f"bps_{qq}")
                        nc.tensor.matmul(
                            st["bps"], ones_row_r, st["rec"], start=True, stop=True
                        )

                    def e_fin():
                        # bps is read straight out of PSUM; the one absorber
                        # transitively clears every epilogue WAR (fin waits
                        # rec/poc which wait dps/po)
                        fin = wkb.tile([P, 512], F32, tag="fin", name=f"fin_{qq}")
                        nc.vector.tensor_mul(out=fin, in0=st["poc"], in1=st["bps"])
                        nc.sync.dma_start(out=outT[:, ts(qq, 512)], in_=fin)
                        pe_absorb(fin)

                    return [e_abs, e_dps, e_rec, e_bps, e_fin]

                def sps_ring(shape, dtype, name):
                    return sp1.tile(shape, dtype, tag="ps", name=name)

                # late-bound ring for epilogues that may spill into pass 2
                # (after the pass-1 PSUM pools close)
                ring_cell = [sps_ring]

                def late_ring(shape, dtype, name):
                    return ring_cell[0](shape, dtype, name)

                # prologue: first two x blocks + k/q projections for block 0
                # (proj_ops(n) prefetches x block n+2); block 0's v
                # projections spill into round 0's background so ACT/DVE
                # start as soon as kT/qT block 0 exist
                emit_xt_dma(0, parts=4)
                if NB > 1:
                    emit_xt_dma(1, parts=4)
                ops0 = proj_ops(0)
                v_spill = ops0[-16:]
                for op in ops0[:-16]:
                    op()

                n_rounds = NB + NFQ - 1
                for n in range(n_rounds):
                    bg = []
                    if n == 0:
                        bg += v_spill
                    if n + 1 < NB:
                        bg += proj_ops(n + 1)
                    for qq in range(NFQ):
                        if n == NB + qq:
                            bg += epilogue_ops(
                                qq, pos[qq], sps_ring, lambda qq=qq: accs[qq]
                            )
                    items = [
                        (qq, n - qq)
                        for qq in range(NFQ)
                        if 0 <= n - qq < NB
                    ]
                    n_iters = 4 * len(items)
                    it = 0
                    for qq, blk in items:
                        ust: dict = {}
                        for j in range(4):
                            # interleave background (projection/epilogue) ops
                            want = (len(bg) * (it + 1)) // n_iters
                            done = (len(bg) * it) // n_iters
                            for _ in range(want - done):
                                bg_op = bg[done]
                                done += 1
                                bg_op()
                            it += 1
                            ck = 4 * blk + j
                            sp = sp1.tile(
                                [P, 512], F32, tag="ps", name=f"sp_{qq}_{ck}"
                            )
                            nc.tensor.matmul(
                                sp,
                                kT[:, ts(ck, P)],
                                qT[:, ts(qq, 512)],
                                start=True,
                                stop=True,
                            )
                            u1 = up.tile(
                                [P, 512], MM_DT, tag="u1", name=f"u_{qq}_{ck}"
                            )
                            nc.scalar.activation(u1, sp, AF.Exp, scale=SM_SCALE)
                            ust[j] = u1
                            if j == 1:
                                fa = fp.tile([P, 512], MM_DT, tag="fA", name=f"fA_{qq}_{blk}")
                                nc.vector.tensor_add(out=fa, in0=ust[0], in1=ust[1])
                                ust["fa"] = fa
                            if j == 3:
                                fb = fp.tile([P, 512], MM_DT, tag="fB", name=f"fB_{qq}_{blk}")
                                nc.vector.tensor_add(out=fb, in0=ust[2], in1=ust[3])
                                fc = fp.tile([P, 512], MM_DT, tag="fC", name=f"fC_{qq}_{blk}")
                                nc.vector.tensor_add(out=fc, in0=ust["fa"], in1=fb)
                                if blk == 0:
                                    acc = accp.tile(
                                        [P, 512], MM_DT, tag=f"acc_{qq}",
                                        name=f"acc_{qq}",
                                    )
                                    accs[qq] = (acc,)
                                    nc.vector.tensor_copy(out=acc, in_=fc)
                                elif blk == NB - 1:
                                    # final add goes to a second tile so the
                                    # epilogue's dacc fold has two operands
                                    accs[qq] = (accs[qq][0], fc)
                                else:
                                    nc.vector.tensor_add(
                                        out=accs[qq][0], in0=accs[qq][0], in1=fc
                                    )
                            pend_pv.append((qq, ck, u1))
                            if len(pend_pv) > PVDEPTH:
                                emit_pv(*pend_pv.pop(0))
                    if n_iters == 0:  # defensive: rounds with no att items
                        for bg_op in bg:
                            bg_op()
                while pend_pv:
                    emit_pv(*pend_pv.pop(0))
                # epilogues not emitted inside rounds spill into pass 2 (or,
                # if there is no pass 2, run here)
                spill: list = []
                for qq in range(NFQ):
                    if NB + qq > n_rounds - 1:
                        spill += epilogue_ops(
                            qq, pos[qq], late_ring, lambda qq=qq: accs[qq]
                        )
                if QNB == NFQ:
                    for op in spill:
                        op()
                    spill = []

            # --- pass 2: remaining query blocks, paired-exp pipeline ---
            if QNB > NFQ:
                with tc.tile_pool(name="psB", bufs=2, space="PSUM") as psB:

                    def psb_ring(shape, dtype, name):
                        return psB.tile(shape, dtype, tag="ps", name=name)

                    absorb_alloc[0] = lambda name: psb_ring([1, 1], F32, name)
                    ring_cell[0] = psb_ring

                    for qb in range(NFQ, QNB):
                        bg2 = spill
                        spill = []
                        po = poB.tile(
                            [P, 512], F32, tag=f"po_{qb % NFQ}", name=f"po2_{qb}"
                        )
                        us: dict[int, object] = {}
                        dacc2 = accp.tile(
                            [P, 1024], MM_DT, tag="dacc2", name=f"dacc2_{qb}"
                        )
                        for mp in range(KP + 1):
                            if bg2:
                                bg2.pop(0)()
                            if mp < KP:
                                sps = psB.tile(
                                    [P, 1024], F32, tag="ps", name=f"sps_{qb}_{mp}"
                                )
                                for h in range(2):
                                    nc.tensor.matmul(
                                        sps[:, ts(h, 512)],
                                        kT[:, ts(2 * mp + h, P)],
                                        qT[:, ts(qb, 512)],
                                        start=True,
                                        stop=True,
                                    )
                                u = up.tile([P, 1024], MM_DT, tag="u", name=f"u2_{qb}_{mp}")
                                nc.scalar.activation(u, sps, AF.Exp, scale=SM_SCALE)
                                us[mp] = u
                                if mp == 0:
                                    nc.vector.tensor_copy(out=dacc2, in_=u)
                                else:
                                    nc.vector.tensor_add(out=dacc2, in0=dacc2, in1=u)
                            if mp > 0:
                                u_prev = us.pop(mp - 1)
                                for h in range(2):
                                    mm = 2 * (mp - 1) + h
                                    nc.tensor.matmul(
                                        po,
                                        vn[:, ts(mm, P)],
                                        u_prev[:, ts(h, 512)],
                                        start=(mm == 0),
                                        stop=(mm == KC - 1),
                                    )
                        for op in epilogue_ops(
                            qb, po, psb_ring,
                            lambda d=dacc2: (d[:, ts(0, 512)], d[:, ts(1, 512)]),
                        ):
                            op()

        if repeat > 1:
            loop_cm.__exit__(None, None, None)

    return nc


_NC_CACHE: dict = {}


def _get_nc(S: int = S_FULL, SQ: int = S_FULL // 2, repeat: int = 1):
    key = (S, SQ, repeat)
    if key not in _NC_CACHE:
        nc = bacc.Bacc("TRN2", debug=False)
        build_attention(nc, S, SQ, repeat)
        nc.compile()  # splits multi-waits into event semaphores (HW limit)
        _NC_CACHE[key] = nc
    return _NC_CACHE[key]


def _bf16(a):
    import ml_dtypes

    return np.ascontiguousarray(np.asarray(a, dtype=np.float32).astype(ml_dtypes.bfloat16))


def make_in_maps(x, Wq, bq, Wk, bk, Wv, bv):
    """Per-core input dicts. Core c = (batch c//2, query-half c%2)."""
    x = np.asarray(x, dtype=np.float32)
    common = {
        "Wq": _bf16(Wq),
        "bq": np.ascontiguousarray(bq, dtype=np.float32),
        "Wk": _bf16(Wk),
        "bk": np.ascontiguousarray(bk, dtype=np.float32),
        "Wv": _bf16(Wv),
        "bv": np.ascontiguousarray(bv, dtype=np.float32),
    }
    in_maps = []
    for c in range(N_CORES):
        b, h = divmod(c, 2)
        xb = x[b]  # [S, D]
        half = S_FULL // 2
        if h == 0:
            perm = xb
        else:
            perm = np.concatenate([xb[half:], xb[:half]], axis=0)
        in_maps.append({"xT": _bf16(perm.T), **common})
    return in_maps


def assemble_output(results):
    """results: list of 8 per-core dicts with 'outT' [128, 2048]."""
    half = S_FULL // 2
    out = np.empty((B, S_FULL, D_QK), dtype=np.float32)
    for c in range(N_CORES):
        b, h = divmod(c, 2)
        out[b, h * half : (h + 1) * half, :] = np.asarray(
            results[c]["outT"], dtype=np.float32
        ).T
    return out


def kernel(x, Wq, bq, Wk, bk, Wv, bv):
    from concourse.bass_utils import run_bass_kernel_spmd

    nc = _get_nc()
    in_maps = make_in_maps(x, Wq, bq, Wk, bk, Wv, bv)
    res = run_bass_kernel_spmd(nc, in_maps, list(range(N_CORES)))
    return assemble_output(res.results)
